# revision 14
# baseline (speedup 1.0000x reference)
"""Trainium2 Bass kernel v13 for nn_AttentionLayer.

Math (per core, vocab-sharded): out[b, v'] = occ[b, v'] * leaky_relu(t[v'] + s[b])
with t = table_shard^T a_w (PE, bf16), s = attr_emb @ a_a (DVE).

v11 vs v9/v10 (~39.5-41.7us). Measured structure: exec ~= first_ACT +
ACT-chain + tail; DMA stream saturates at ~0.4 MB/us and is not the
binding constraint once bytes are ~8.2 MB. So:
  - strip 0 is only 512 wide: its table chunk (262 KB) lands ~11.5us and
    two cold matmuls later the ACT chain starts ~13 (was 17.6-19.5).
  - remaining 4 strips of 1472 (3 PSUM banks x 2 bufs + 1 bank for
    strip 0 = 7 of 8 banks).
  - no PE warm-up (never flipped HAM in time), no split tail (v10 showed
    both regress).
  - aa/attr in bf16 - lighter and lands earlier for the s bias.
HBM/core: tbl 3.28 + occ 1.64 (i8) + out 3.28 + 0.2 smalls = 8.4 MB.
"""

import numpy as np
import ml_dtypes

import concourse.bass as bass
import concourse.tile as tile
from concourse import bacc, mybir
from concourse.bass_utils import run_bass_kernel_spmd

B = 256
L = 512
V = 50257
DW = 256
DA = 256
ALPHA = 0.2

NCORES = 8
VS = 6400
WIDTHS = (512, 1024, 1024, 1024, 1024, 1024, 768)   # strip widths, sum = VS
NS = len(WIDTHS)
OFFS = tuple(np.cumsum((0,) + WIDTHS).tolist())  # col offsets, len NS+1
OCC_SPLIT = 3584                          # occ chunk boundary = OFFS[4]

BF16 = ml_dtypes.bfloat16

_CACHE = {}


def _nchunks(w):
    c, n0 = [], 0
    while n0 < w:
        n1 = min(n0 + 512, w)
        c.append((n0, n1))
        n0 = n1
    return tuple(c)


def _build():
    if "nc" in _CACHE:
        return _CACHE["nc"]
    f32 = mybir.dt.float32
    bf16 = mybir.dt.bfloat16
    i8 = mybir.dt.int8

    nc = bacc.Bacc("TRN2", target_bir_lowering=False, debug=False)
    tbl = nc.declare_dram_parameter("tbl", [128, 2 * VS], bf16, isOutput=False)
    occ = nc.declare_dram_parameter("occ", [128, 2 * VS], bf16, isOutput=False)
    # packed small weights: [awb(256) | aa(256) | attr(512)]
    wsm = nc.declare_dram_parameter("wsm", [128, 1024], bf16, isOutput=False)
    out = nc.declare_dram_parameter("out", [128, 2 * VS], bf16, isOutput=True)

    with tile.TileContext(nc) as tc:
        with (
            tc.tile_pool(name="sb", bufs=1) as sb,
            tc.tile_pool(name="ps0", bufs=1, space="PSUM") as ps0,
            tc.tile_pool(name="psm", bufs=3, space="PSUM") as psm,
        ):
            # ---- one packed small-weight load, FIRST on the sync ring;
            # everything (LDWEIGHTS, s-prep) keys off this single landing.
            # scalar ring carries nothing: the ACT chain never stalls on a
            # DMA ring slot. ----
            wsm_t = sb.tile([128, 1024], bf16, tag="wsm")
            nc.sync.dma_start(wsm_t[:], wsm.ap())
            awb_t = wsm_t[:, 0:256]
            aa_t = wsm_t[:, 256:512]
            at = wsm_t[:, 512:1024]

            # ---- sync ring: per-strip table chunks ([dh0|dh1] interleaved
            # by host) with occ chunks slotted between ----
            ts = {}
            ocg = [[None, None], [None, None]]
            occ_bounds = ((0, OCC_SPLIT), (OCC_SPLIT, VS))
            for si in range(NS):
                w = WIDTHS[si]
                t_ = sb.tile([128, 2 * w], bf16, tag=f"t{si}", name=f"t{si}")
                nc.sync.dma_start(
                    t_[:], tbl.ap()[:, 2 * OFFS[si] : 2 * OFFS[si + 1]]
                )
                ts[si] = t_
                if si == 3:
                    for h in range(2):
                        c0, c1 = occ_bounds[0]
                        o_ = sb.tile([128, c1 - c0], bf16, tag=f"oc{h}0", name=f"oc{h}0")
                        nc.sync.dma_start(
                            o_[:], occ.ap()[:, h * VS + c0 : h * VS + c1]
                        )
                        ocg[h][0] = o_
                if si == NS - 1:
                    for h in range(2):
                        c0, c1 = occ_bounds[1]
                        o_ = sb.tile([128, c1 - c0], bf16, tag=f"oc{h}1", name=f"oc{h}1")
                        nc.sync.dma_start(
                            o_[:], occ.ap()[:, h * VS + c0 : h * VS + c1]
                        )
                        ocg[h][1] = o_

            # ---- s = attr_emb @ a_a ----
            s_sb = sb.tile([128, 2], f32, tag="s")
            for h in range(2):
                pa = sb.tile([128, DA], f32, tag=f"pa{h}")
                nc.vector.tensor_tensor(
                    out=pa[:],
                    in0=at[:, h * DA : (h + 1) * DA],
                    in1=aa_t,
                    op=mybir.AluOpType.mult,
                )
                nc.vector.tensor_reduce(
                    out=s_sb[:, h : h + 1],
                    in_=pa[:],
                    axis=mybir.AxisListType.X,
                    op=mybir.AluOpType.add,
                )

            # ---- per strip: matmul, ACT x2, mask x2, store x2 ----
            for si in range(NS):
                w = WIDTHS[si]
                grp = 0 if OFFS[si] < OCC_SPLIT else 1
                off = OFFS[si] - (0 if grp == 0 else OCC_SPLIT)
                if si == 0:
                    pt = ps0.tile([128, w], f32, tag="pt0", name=f"pt{si}")
                else:
                    ptbuf = psm.tile([128, 1024], f32, tag="pt", name=f"pt{si}")
                    pt = ptbuf[:, 0:w]
                for dh in range(2):
                    for n0, n1 in _nchunks(w):
                        nc.tensor.matmul(
                            pt[:, n0:n1],
                            lhsT=awb_t[:, dh * 128 : (dh + 1) * 128],
                            rhs=ts[si][:, dh * w + n0 : dh * w + n1],
                            start=(dh == 0),
                            stop=(dh == 1),
                        )
                for h in range(2):
                    o1 = sb.tile([128, w], bf16, tag=f"o1_{si}_{h}", name=f"o1_{si}_{h}")
                    nc.scalar.activation(
                        o1[:],
                        pt[:],
                        mybir.ActivationFunctionType.Prelu,
                        bias=s_sb[:, h : h + 1],
                        scale=1.0,
                        alpha=ALPHA,
                    )
                    o = sb.tile([128, w], bf16, tag=f"o_{si}_{h}", name=f"o_{si}_{h}")
                    nc.vector.tensor_tensor(
                        out=o[:],
                        in0=o1[:],
                        in1=ocg[h][grp][:, off : off + w],
                        op=mybir.AluOpType.mult,
                    )
                    nc.sync.dma_start(
                        out.ap()[:, h * VS + OFFS[si] : h * VS + OFFS[si + 1]],
                        o[:],
                    )

    nc.compile()
    _CACHE["nc"] = nc
    return nc


def _pmaj(x):
    """[256, N] -> partition-major [128, 2*N] (halves along columns)."""
    n = x.shape[1]
    return np.ascontiguousarray(
        x.reshape(2, 128, n).transpose(1, 0, 2).reshape(128, 2 * n)
    )


def _prep_inputs(words, word_emb_table, attr_emb, a):
    words = np.ascontiguousarray(words).astype(np.int64)
    wet = np.ascontiguousarray(word_emb_table, dtype=np.float32)
    attr = np.ascontiguousarray(attr_emb, dtype=np.float32)
    a = np.ascontiguousarray(a, dtype=np.float32).reshape(-1)

    A = a[:DW].astype(BF16).reshape(2, 128)
    awb_dev = np.repeat(A.T[:, :, None], 128, axis=2).reshape(128, 2 * 128)
    aa_rep = np.broadcast_to(a[DW:].astype(BF16)[None, :], (128, DA))
    attr_dev = _pmaj(attr.astype(BF16))
    wsm_dev = np.ascontiguousarray(
        np.concatenate([awb_dev, aa_rep, attr_dev], axis=1)
    )

    tblpad = np.zeros((NCORES * VS, DW), dtype=np.float32)
    tblpad[:V] = wet
    tbl_bf = tblpad.astype(BF16)

    occ_full = np.zeros((B, NCORES * VS), dtype=BF16)
    rows = np.repeat(np.arange(B), L)
    occ_full[rows, words.reshape(-1)] = 1

    in_maps = []
    for i in range(NCORES):
        blk = tbl_bf[i * VS : (i + 1) * VS, :]          # [VS, 256]
        bt = blk.T.reshape(2, 128, VS)                  # [dh, p, v]
        # per-strip contiguous [dh0 cols | dh1 cols] chunks
        cols = []
        for si in range(NS):
            seg = bt[:, :, OFFS[si] : OFFS[si + 1]]     # [2, 128, w]
            cols.append(seg.transpose(1, 0, 2).reshape(128, -1))
        tbl_dev = np.ascontiguousarray(np.concatenate(cols, axis=1))
        occ_dev = _pmaj(occ_full[:, i * VS : (i + 1) * VS])
        in_maps.append(
            {"tbl": tbl_dev, "occ": occ_dev, "wsm": wsm_dev}
        )
    return in_maps


def kernel(words, word_emb_table, attr_emb, a, _trace=False, **_kw):
    nc = _build()
    in_maps = _prep_inputs(words, word_emb_table, attr_emb, a)
    res = run_bass_kernel_spmd(nc, in_maps, list(range(NCORES)), trace=_trace)
    parts = []
    for i in range(NCORES):
        o = res.results[i]["out"]                       # [128, 2*VS] bf16
        parts.append(o.reshape(128, 2, VS).transpose(1, 0, 2).reshape(B, VS))
    out = np.ascontiguousarray(
        np.concatenate(parts, axis=1)[:, :V].astype(np.float32)
    )
    if _trace:
        return out, res
    return out


# revision 15
# speedup vs baseline: 1.1227x; 1.1227x over previous
"""Trainium2 Bass kernel v14 for nn_AttentionLayer.

Math (per core, vocab-sharded): out[b, v'] = occ[b, v'] * leaky_relu(t[v'] + s[b])
with t = table_shard^T a_w (PE, bf16), s = attr_emb @ a_a (DVE).

v11 vs v9/v10 (~39.5-41.7us). Measured structure: exec ~= first_ACT +
ACT-chain + tail; DMA stream saturates at ~0.4 MB/us and is not the
binding constraint once bytes are ~8.2 MB. So:
  - strip 0 is only 512 wide: its table chunk (262 KB) lands ~11.5us and
    two cold matmuls later the ACT chain starts ~13 (was 17.6-19.5).
  - remaining 4 strips of 1472 (3 PSUM banks x 2 bufs + 1 bank for
    strip 0 = 7 of 8 banks).
  - no PE warm-up (never flipped HAM in time), no split tail (v10 showed
    both regress).
  - aa/attr in bf16 - lighter and lands earlier for the s bias.
HBM/core: tbl 3.28 + occ 1.64 (i8) + out 3.28 + 0.2 smalls = 8.4 MB.
"""

import numpy as np
import ml_dtypes

import concourse.bass as bass
import concourse.tile as tile
from concourse import bacc, mybir
from concourse.bass_utils import run_bass_kernel_spmd

B = 256
L = 512
V = 50257
DW = 256
DA = 256
ALPHA = 0.2

NCORES = 8
VS = 6400
WIDTHS = (512, 1024, 1024, 1024, 1024, 1024, 768)   # strip widths, sum = VS
NS = len(WIDTHS)
OFFS = tuple(np.cumsum((0,) + WIDTHS).tolist())  # col offsets, len NS+1
OCC_SPLIT = 3584                          # occ chunk boundary = OFFS[4]

BF16 = ml_dtypes.bfloat16

_CACHE = {}


def _nchunks(w):
    c, n0 = [], 0
    while n0 < w:
        n1 = min(n0 + 512, w)
        c.append((n0, n1))
        n0 = n1
    return tuple(c)


def _build():
    if "nc" in _CACHE:
        return _CACHE["nc"]
    f32 = mybir.dt.float32
    bf16 = mybir.dt.bfloat16
    i8 = mybir.dt.int8

    nc = bacc.Bacc("TRN2", target_bir_lowering=False, debug=False)
    tbl = nc.declare_dram_parameter("tbl", [128, 2 * VS], bf16, isOutput=False)
    occ = nc.declare_dram_parameter("occ", [128, 2 * VS], bf16, isOutput=False)
    # packed small weights: [awb(256) | aa(256) | attr(512)]
    wsm = nc.declare_dram_parameter("wsm", [128, 1024], bf16, isOutput=False)
    out = nc.declare_dram_parameter("out", [128, 2 * VS], bf16, isOutput=True)

    with tile.TileContext(nc) as tc:
        with (
            tc.tile_pool(name="sb", bufs=1) as sb,
            tc.tile_pool(name="ps0", bufs=1, space="PSUM") as ps0,
            tc.tile_pool(name="psm", bufs=3, space="PSUM") as psm,
        ):
            # ---- one packed small-weight load, FIRST on the sync ring;
            # everything (LDWEIGHTS, s-prep) keys off this single landing.
            # scalar ring carries nothing: the ACT chain never stalls on a
            # DMA ring slot. ----
            wsm_t = sb.tile([128, 1024], bf16, tag="wsm")
            nc.sync.dma_start(wsm_t[:], wsm.ap())
            awb_t = wsm_t[:, 0:256]
            aa_t = wsm_t[:, 256:512]
            at = wsm_t[:, 512:1024]

            # ---- sync ring: per-strip table chunks ([dh0|dh1] interleaved
            # by host) with occ chunks slotted between ----
            ts = {}
            ocg = [[None, None], [None, None]]
            occ_bounds = ((0, OCC_SPLIT), (OCC_SPLIT, VS))

            def load_t(si):
                w = WIDTHS[si]
                t_ = sb.tile([128, 2 * w], bf16, tag=f"t{si}", name=f"t{si}")
                nc.sync.dma_start(
                    t_[:], tbl.ap()[:, 2 * OFFS[si] : 2 * OFFS[si + 1]]
                )
                ts[si] = t_

            def load_oc(h, grp):
                c0, c1 = occ_bounds[grp]
                o_ = sb.tile([128, c1 - c0], bf16, tag=f"oc{h}{grp}",
                             name=f"oc{h}{grp}")
                nc.sync.dma_start(o_[:], occ.ap()[:, h * VS + c0 : h * VS + c1])
                ocg[h][grp] = o_

            # keep the PE fed first (t0-t4), then slot occ chunks so each
            # lands just before its consumers; oc[1][1] (the terminal TT's
            # input) goes last but still ahead of when it is needed.
            for si in range(5):
                load_t(si)
            load_oc(0, 0)
            load_t(5)
            load_t(6)
            load_oc(1, 0)
            load_oc(0, 1)
            load_oc(1, 1)

            # ---- s = attr_emb @ a_a ----
            s_sb = sb.tile([128, 2], f32, tag="s")
            for h in range(2):
                pa = sb.tile([128, DA], f32, tag=f"pa{h}")
                nc.vector.tensor_tensor(
                    out=pa[:],
                    in0=at[:, h * DA : (h + 1) * DA],
                    in1=aa_t,
                    op=mybir.AluOpType.mult,
                )
                nc.vector.tensor_reduce(
                    out=s_sb[:, h : h + 1],
                    in_=pa[:],
                    axis=mybir.AxisListType.X,
                    op=mybir.AluOpType.add,
                )

            # ---- per strip: matmul, ACT x2, mask x2, store x2 ----
            for si in range(NS):
                w = WIDTHS[si]
                grp = 0 if OFFS[si] < OCC_SPLIT else 1
                off = OFFS[si] - (0 if grp == 0 else OCC_SPLIT)
                if si == 0:
                    pt = ps0.tile([128, w], f32, tag="pt0", name=f"pt{si}")
                else:
                    ptbuf = psm.tile([128, 1024], f32, tag="pt", name=f"pt{si}")
                    pt = ptbuf[:, 0:w]
                for dh in range(2):
                    for n0, n1 in _nchunks(w):
                        nc.tensor.matmul(
                            pt[:, n0:n1],
                            lhsT=awb_t[:, dh * 128 : (dh + 1) * 128],
                            rhs=ts[si][:, dh * w + n0 : dh * w + n1],
                            start=(dh == 0),
                            stop=(dh == 1),
                        )
                for h in range(2):
                    o1 = sb.tile([128, w], bf16, tag=f"o1_{si}_{h}", name=f"o1_{si}_{h}")
                    nc.scalar.activation(
                        o1[:],
                        pt[:],
                        mybir.ActivationFunctionType.Prelu,
                        bias=s_sb[:, h : h + 1],
                        scale=1.0,
                        alpha=ALPHA,
                    )
                    o = sb.tile([128, w], bf16, tag=f"o_{si}_{h}", name=f"o_{si}_{h}")
                    nc.vector.tensor_tensor(
                        out=o[:],
                        in0=o1[:],
                        in1=ocg[h][grp][:, off : off + w],
                        op=mybir.AluOpType.mult,
                    )
                    nc.sync.dma_start(
                        out.ap()[:, h * VS + OFFS[si] : h * VS + OFFS[si + 1]],
                        o[:],
                    )

    nc.compile()
    _CACHE["nc"] = nc
    return nc


def _pmaj(x):
    """[256, N] -> partition-major [128, 2*N] (halves along columns)."""
    n = x.shape[1]
    return np.ascontiguousarray(
        x.reshape(2, 128, n).transpose(1, 0, 2).reshape(128, 2 * n)
    )


def _prep_inputs(words, word_emb_table, attr_emb, a):
    words = np.ascontiguousarray(words).astype(np.int64)
    wet = np.ascontiguousarray(word_emb_table, dtype=np.float32)
    attr = np.ascontiguousarray(attr_emb, dtype=np.float32)
    a = np.ascontiguousarray(a, dtype=np.float32).reshape(-1)

    A = a[:DW].astype(BF16).reshape(2, 128)
    awb_dev = np.repeat(A.T[:, :, None], 128, axis=2).reshape(128, 2 * 128)
    aa_rep = np.broadcast_to(a[DW:].astype(BF16)[None, :], (128, DA))
    attr_dev = _pmaj(attr.astype(BF16))
    wsm_dev = np.ascontiguousarray(
        np.concatenate([awb_dev, aa_rep, attr_dev], axis=1)
    )

    tblpad = np.zeros((NCORES * VS, DW), dtype=np.float32)
    tblpad[:V] = wet
    tbl_bf = tblpad.astype(BF16)

    occ_full = np.zeros((B, NCORES * VS), dtype=BF16)
    rows = np.repeat(np.arange(B), L)
    occ_full[rows, words.reshape(-1)] = 1

    in_maps = []
    for i in range(NCORES):
        blk = tbl_bf[i * VS : (i + 1) * VS, :]          # [VS, 256]
        bt = blk.T.reshape(2, 128, VS)                  # [dh, p, v]
        # per-strip contiguous [dh0 cols | dh1 cols] chunks
        cols = []
        for si in range(NS):
            seg = bt[:, :, OFFS[si] : OFFS[si + 1]]     # [2, 128, w]
            cols.append(seg.transpose(1, 0, 2).reshape(128, -1))
        tbl_dev = np.ascontiguousarray(np.concatenate(cols, axis=1))
        occ_dev = _pmaj(occ_full[:, i * VS : (i + 1) * VS])
        in_maps.append(
            {"tbl": tbl_dev, "occ": occ_dev, "wsm": wsm_dev}
        )
    return in_maps


def kernel(words, word_emb_table, attr_emb, a, _trace=False, **_kw):
    nc = _build()
    in_maps = _prep_inputs(words, word_emb_table, attr_emb, a)
    res = run_bass_kernel_spmd(nc, in_maps, list(range(NCORES)), trace=_trace)
    parts = []
    for i in range(NCORES):
        o = res.results[i]["out"]                       # [128, 2*VS] bf16
        parts.append(o.reshape(128, 2, VS).transpose(1, 0, 2).reshape(B, VS))
    out = np.ascontiguousarray(
        np.concatenate(parts, axis=1)[:, :V].astype(np.float32)
    )
    if _trace:
        return out, res
    return out
